# revision 1
# baseline (speedup 1.0000x reference)
"""Trainium2 Bass kernel for a dense decoder block (LN->MHA->res, LN->FFN->res).

Sharding (8 cores, one NEFF, SPMD-uniform addressing):
  - LN1 token-parallel (512-token chunk/core) -> AllGather of normalized acts.
  - QKV + attention head-parallel (2 heads/core, causal, unstable softmax --
    exact because masked logits multiply to 0 post-exp).
  - AllToAll redistributes attention values: head-shards -> token-shards.
  - proj + residual + LN2 + FFN token-parallel with full weights streamed.
  - LN affine params are folded into the following matmul weights on host.

All activations stay channel-major [C, tokens] on device so the whole matmul
chain needs zero transposes: weights ride as stationary lhsT, activations
stream as rhs, outputs land channel-major in PSUM. Matmuls run as float32r
(fp32 storage, ~12-bit mantissa in the PE, full speed at N>=256).
"""

import math

import numpy as np

import concourse.bass as bass
import concourse.mybir as mybir
import concourse.tile as tile
from concourse import bacc
from concourse import bass_utils

F32 = mybir.dt.float32
F32R = mybir.dt.float32r
AF = mybir.ActivationFunctionType
OP = mybir.AluOpType

N_CORES = 8
B = 2
C = 2048
H = 16
HD = 128
F = 8192
H_PER_CORE = H // N_CORES          # 2
NCT = C // 128                     # 16 channel tiles
NFT = F // 128                     # 64 ffn tiles
EPS = 1e-5
SCALE = 1.0 / math.sqrt(HD)
GELU = AF.Gelu_apprx_tanh  # swapped to a CoreSim-implemented func in sim tests


def r32(ap):
    return ap.bitcast(F32R)


def _ln_stats_mm(nc, ps_sum, ps_ssq, ones_sq, x_tile, sq_tile, k, nk):
    """Accumulate broadcast column sums of x and x^2 into [128, N] psums."""
    nc.scalar.activation(sq_tile[:], x_tile, AF.Square)
    nc.tensor.matmul(ps_sum[:], ones_sq[:], x_tile,
                     start=(k == 0), stop=(k == nk - 1))
    nc.tensor.matmul(ps_ssq[:], ones_sq[:], sq_tile[:],
                     start=(k == 0), stop=(k == nk - 1))


def _ln_finish(nc, pool_small, ps_sum, ps_ssq, n_tok, ncols):
    """From broadcast sum/sumsq psums produce SBUF rstd/shift [128, ncols]."""
    mean = pool_small.tile([128, ncols], F32, tag="ln_mean")
    ex2 = pool_small.tile([128, ncols], F32, tag="ln_ex2")
    nc.vector.tensor_scalar_mul(mean[:], ps_sum[:], 1.0 / n_tok)
    nc.vector.tensor_scalar_mul(ex2[:], ps_ssq[:], 1.0 / n_tok)
    msq = pool_small.tile([128, ncols], F32, tag="ln_msq")
    nc.vector.tensor_mul(msq[:], mean[:], mean[:])
    varp = pool_small.tile([128, ncols], F32, tag="ln_varp")
    # (ex2 + eps) - mean^2
    nc.vector.scalar_tensor_tensor(varp[:], ex2[:], EPS, msq[:],
                                   op0=OP.add, op1=OP.subtract)
    std = pool_small.tile([128, ncols], F32, tag="ln_std")
    nc.scalar.sqrt(std[:], varp[:])
    rstd_bc = pool_small.tile([128, ncols], F32, tag="ln_rstd")
    nc.vector.reciprocal(rstd_bc[:], std[:])
    shift_bc = pool_small.tile([128, ncols], F32, tag="ln_shift")
    # (mean * -1) * rstd
    nc.vector.scalar_tensor_tensor(shift_bc[:], mean[:], -1.0, rstd_bc[:],
                                   op0=OP.mult, op1=OP.mult)
    return rstd_bc, shift_bc


def build_decoder(T=2048, collectives=True, debug=False):
    """Build the SPMD decoder-block program for seq length T (2048 = real)."""
    NT = B * T                      # total tokens
    CH = NT // N_CORES              # tokens per core chunk
    NQS = T // 512 if T >= 512 else 1   # tq slices of 512 per batch elem
    QS = min(512, T)                # tq slice width
    NKT = T // 128                  # tk tiles per batch elem
    S_SUB = CH // 128               # 128-token subtiles per chunk
    n_chunks = N_CORES

    nc = bacc.Bacc("TRN2", target_bir_lowering=False, debug=False,
                   num_devices=N_CORES)

    # ---- I/O ----
    xt = nc.dram_tensor("xt", [C, CH], F32, kind="ExternalInput").ap()
    wq = nc.dram_tensor("wq", [C, 256], F32R, kind="ExternalInput").ap()
    wk = nc.dram_tensor("wk", [C, 256], F32R, kind="ExternalInput").ap()
    wv = nc.dram_tensor("wv", [C, 256], F32R, kind="ExternalInput").ap()
    bq = nc.dram_tensor("bq", [256, 1], F32, kind="ExternalInput").ap()
    bk = nc.dram_tensor("bk", [256, 1], F32, kind="ExternalInput").ap()
    bv_bc = nc.dram_tensor("bv_bc", [128, 256], F32, kind="ExternalInput").ap()
    wproj = nc.dram_tensor("wproj", [C, C], F32R, kind="ExternalInput").ap()
    bproj = nc.dram_tensor("bproj", [C, 1], F32, kind="ExternalInput").ap()
    wf1t = nc.dram_tensor("wf1t", [NFT, C, 128], F32R, kind="ExternalInput").ap()
    bf1 = nc.dram_tensor("bf1", [F, 1], F32, kind="ExternalInput").ap()
    wf2 = nc.dram_tensor("wf2", [F, C], F32R, kind="ExternalInput").ap()
    bf2 = nc.dram_tensor("bf2", [C, 1], F32, kind="ExternalInput").ap()
    masks = nc.dram_tensor("masks", [128, 4, QS], F32R, kind="ExternalInput").ap()
    out = nc.dram_tensor("out", [C, CH], F32, kind="ExternalOutput").ap()
    if debug:
        dbg_n1 = nc.dram_tensor("dbg_n1", [N_CORES * C, CH], F32R, kind="ExternalOutput").ap()
        dbg_q = nc.dram_tensor("dbg_q", [128, H_PER_CORE * B * T], F32R, kind="ExternalOutput").ap()
        dbg_k = nc.dram_tensor("dbg_k", [128, H_PER_CORE * B * T], F32R, kind="ExternalOutput").ap()
        dbg_v = nc.dram_tensor("dbg_v", [128, (B * T // 128) * 256], F32R, kind="ExternalOutput").ap()
        dbg_vals = nc.dram_tensor("dbg_vals", [C, CH], F32R, kind="ExternalOutput").ap()
        dbg_pv = nc.dram_tensor("dbg_pv", [C, CH], F32R, kind="ExternalOutput").ap()
        dbg_r1 = nc.dram_tensor("dbg_r1", [C, CH], F32, kind="ExternalOutput").ap()

    RG = [list(range(N_CORES))]

    with tile.TileContext(nc) as tc:
        with tc.tile_pool(name="dram", bufs=1, space="DRAM") as dram, \
             tc.tile_pool(name="persist", bufs=1) as persist:
            n1_bounce = dram.tile([C, CH], F32R, tag="n1_bounce")
            n1_full = dram.tile([N_CORES * C, CH], F32R, tag="n1_full",
                                addr_space="Shared")
            a2a_in = dram.tile([C, CH], F32R, tag="a2a_in")
            a2a_out = dram.tile([C, CH], F32R, tag="a2a_out")
            r1_dram = dram.tile([C, CH], F32, tag="r1_dram")

            ones_sq = persist.tile([128, 128], F32, tag="ones_sq")
            ones_sq_r = persist.tile([128, 128], F32R, tag="ones_sq_r")
            nc.vector.memset(ones_sq[:], 1.0)
            nc.vector.tensor_copy(ones_sq_r[:], ones_sq[:])
            masks_sb = persist.tile([128, 4, QS], F32R, tag="masks")
            nc.sync.dma_start(masks_sb[:], masks)
            bq_sb = persist.tile([128, 2, 1], F32, tag="bq")
            bk_sb = persist.tile([128, 2, 1], F32, tag="bk")
            nc.sync.dma_start(bq_sb[:], bq.rearrange("(o p) u -> p o u", p=128))
            nc.sync.dma_start(bk_sb[:], bk.rearrange("(o p) u -> p o u", p=128))
            bv_sb = persist.tile([128, 256], F32, tag="bv")
            nc.sync.dma_start(bv_sb[:], bv_bc)
            bproj_sb = persist.tile([128, NCT, 1], F32, tag="bproj")
            nc.sync.dma_start(bproj_sb[:], bproj.rearrange("(o p) u -> p o u", p=128))
            bf1_sb = persist.tile([128, NFT, 1], F32, tag="bf1")
            nc.sync.dma_start(bf1_sb[:], bf1.rearrange("(o p) u -> p o u", p=128))
            bf2_sb = persist.tile([128, NCT, 1], F32, tag="bf2")
            nc.sync.dma_start(bf2_sb[:], bf2.rearrange("(o p) u -> p o u", p=128))

            # ================= Phase A: LN1 on own chunk =================
            with tc.tile_pool(name="lnA", bufs=2) as lnA, \
                 tc.tile_pool(name="lnA_small", bufs=1) as lnAs, \
                 tc.tile_pool(name="n1pool", bufs=1) as n1pool, \
                 tc.tile_pool(name="psA", bufs=1, space="PSUM") as psA:
                xt_view = xt.rearrange("(k p) t -> p k t", p=128)
                x_sb = n1pool.tile([128, NCT, CH], F32, tag="x_sb")
                nc.sync.dma_start(x_sb[:], xt_view)
                ps_sum = psA.tile([128, CH], F32, tag="sum")
                ps_ssq = psA.tile([128, CH], F32, tag="ssq")
                for k in range(NCT):
                    sq = lnA.tile([128, CH], F32, tag="sq")
                    _ln_stats_mm(nc, ps_sum, ps_ssq, ones_sq,
                                 x_sb[:, k, :], sq, k, NCT)
                rstd_bc, shift_bc = _ln_finish(nc, lnAs, ps_sum, ps_ssq, C, CH)
                n1_sb = n1pool.tile([128, NCT, CH], F32R, tag="n1_sb")
                for k in range(NCT):
                    nc.vector.tensor_mul(n1_sb[:, k, :], x_sb[:, k, :], rstd_bc[:])
                    nc.vector.tensor_add(n1_sb[:, k, :], n1_sb[:, k, :], shift_bc[:])
                nc.sync.dma_start(n1_bounce[:].rearrange("(k p) t -> p k t", p=128),
                                  n1_sb[:])

            if collectives:
                nc.gpsimd.collective_compute(
                    "AllGather", OP.bypass, replica_groups=RG,
                    ins=[n1_bounce.opt()], outs=[n1_full.opt()])
            else:  # timing variant: plain copy keeps the dependency edge
                nc.sync.dma_start(n1_full[0:C, :], n1_bounce[:])

            # ============ Phase B: QKV (all tokens, own 2 heads) ============
            q_cols = H_PER_CORE * T
            with tc.tile_pool(name="wqkv", bufs=1) as wqkvp, \
                 tc.tile_pool(name="qkv_sb", bufs=1) as qkvp:
                wq_sb = wqkvp.tile([128, NCT, 256], F32R, tag="wq")
                wk_sb = wqkvp.tile([128, NCT, 256], F32R, tag="wk")
                wv_sb = wqkvp.tile([128, NCT, 256], F32R, tag="wv")
                nc.sync.dma_start(wq_sb[:], wq.rearrange("(k p) m -> p k m", p=128))
                nc.sync.dma_start(wk_sb[:], wk.rearrange("(k p) m -> p k m", p=128))
                nc.sync.dma_start(wv_sb[:], wv.rearrange("(k p) m -> p k m", p=128))
                q_sb = qkvp.tile([128, H_PER_CORE * B * T], F32R, tag="q_sb")
                k_sb = qkvp.tile([128, H_PER_CORE * B * T], F32R, tag="k_sb")
                v_sb = qkvp.tile([128, NT // 128, 256], F32R, tag="v_sb")

                with tc.tile_pool(name="n1t", bufs=4) as n1tp, \
                     tc.tile_pool(name="psQK", bufs=1, space="PSUM") as psQK, \
                     tc.tile_pool(name="psV", bufs=1, space="PSUM") as psV:
                    for r in range(n_chunks):
                        ps_q = [psQK.tile([128, CH], F32, tag=f"q{o}", name=f"ps_q{o}") for o in range(2)]
                        ps_k = [psQK.tile([128, CH], F32, tag=f"k{o}", name=f"ps_k{o}") for o in range(2)]
                        ps_v = [psV.tile([128, 256], F32, tag=f"v{s}", name=f"ps_v{s}")
                                for s in range(S_SUB)]
                        for k in range(NCT):
                            n1t = n1tp.tile([128, CH], F32R, tag="n1t")
                            nc.sync.dma_start(
                                n1t[:], n1_full[C * r + 128 * k: C * r + 128 * (k + 1), :])
                            for o in range(2):
                                nc.tensor.matmul(
                                    ps_q[o][:], wq_sb[:, k, 128 * o:128 * (o + 1)],
                                    n1t[:], start=(k == 0), stop=(k == NCT - 1))
                                nc.tensor.matmul(
                                    ps_k[o][:], wk_sb[:, k, 128 * o:128 * (o + 1)],
                                    n1t[:], start=(k == 0), stop=(k == NCT - 1))
                            for s in range(S_SUB):
                                nc.tensor.matmul(
                                    ps_v[s][:],
                                    n1t[:, 128 * s:128 * (s + 1)],
                                    wv_sb[:, k, :],
                                    start=(k == 0), stop=(k == NCT - 1))
                        # drains: q/k head h slice for tokens of chunk r
                        for o in range(2):
                            nc.scalar.activation(
                                q_sb[:, _qk_col(o, r, T, CH):_qk_col(o, r, T, CH) + CH],
                                ps_q[o][:], AF.Identity, bias=bq_sb[:, o, :], scale=1.0)
                            nc.scalar.activation(
                                k_sb[:, _qk_col(o, r, T, CH):_qk_col(o, r, T, CH) + CH],
                                ps_k[o][:], AF.Identity, bias=bk_sb[:, o, :], scale=1.0)
                        for s in range(S_SUB):
                            tt = r * S_SUB + s
                            nc.vector.tensor_add(v_sb[:, tt, :], ps_v[s][:],
                                                 bv_sb[:])

                if debug:
                    nc.sync.dma_start(dbg_q, q_sb[:])
                    nc.sync.dma_start(dbg_k, k_sb[:])
                    nc.sync.dma_start(dbg_v, v_sb[:].rearrange("p a m -> p (a m)"))
                # ============ Phase B2: attention per (head, batch) ============
                with tc.tile_pool(name="attn_e", bufs=5) as ep, \
                     tc.tile_pool(name="attn_small", bufs=3) as asml, \
                     tc.tile_pool(name="vals", bufs=2) as valsp, \
                     tc.tile_pool(name="psS", bufs=3, space="PSUM") as psS, \
                     tc.tile_pool(name="psAV", bufs=2, space="PSUM") as psAV, \
                     tc.tile_pool(name="psDen", bufs=2, space="PSUM") as psDen, \
                     tc.tile_pool(name="psBC", bufs=1, space="PSUM") as psBC:
                    for h in range(H_PER_CORE):
                        for bb in range(B):
                            for j in range(NQS):
                                ni = 4 * (j + 1) if QS == 512 else NKT
                                ps_av = psAV.tile([128, QS], F32, tag="av")
                                ps_den = psDen.tile([128, QS], F32, tag="den")
                                qcol = _qk_col_tok(h, bb * T + j * QS, T, CH)
                                for i in range(ni):
                                    ps_s = psS.tile([128, QS], F32, tag="s")
                                    kcol = _qk_col_tok(h, bb * T + i * 128, T, CH)
                                    nc.tensor.matmul(
                                        ps_s[:], k_sb[:, kcol:kcol + 128],
                                        q_sb[:, qcol:qcol + QS],
                                        start=True, stop=True)
                                    e = ep.tile([128, QS], F32R, tag="e")
                                    nc.scalar.activation(e[:], ps_s[:], AF.Exp,
                                                         bias=0.0, scale=SCALE)
                                    di = i - (ni - 4)
                                    if di >= 0:
                                        nc.vector.tensor_mul(
                                            e[:], e[:], masks_sb[:, di, :])
                                    nc.tensor.matmul(
                                        ps_den[:], ones_sq_r[:], e[:],
                                        start=(i == 0), stop=(i == ni - 1))
                                    tt = (bb * T + i * 128) // 128
                                    nc.tensor.matmul(
                                        ps_av[:],
                                        v_sb[:, tt, h * 128:(h + 1) * 128],
                                        e[:],
                                        start=(i == 0), stop=(i == ni - 1))
                                rec_bc = asml.tile([128, QS], F32, tag="rec_bc")
                                nc.vector.reciprocal(rec_bc[:], ps_den[:])
                                vtile = valsp.tile([128, QS], F32R, tag="vt")
                                nc.vector.tensor_mul(vtile[:], ps_av[:], rec_bc[:])
                                # DMA into a2a_in shards: rows [256*jg+128*h : +128]
                                ncol0 = bb * T + j * QS
                                for part in range(max(1, QS // CH)):
                                    jg = (ncol0 + part * CH) // CH
                                    w = min(CH, QS)
                                    nc.sync.dma_start(
                                        a2a_in[256 * jg + 128 * h:
                                               256 * jg + 128 * (h + 1), :],
                                        vtile[:, part * w:(part + 1) * w])

            if collectives:
                nc.gpsimd.collective_compute(
                    "AllToAll", OP.bypass, replica_groups=RG,
                    ins=[a2a_in.opt()], outs=[a2a_out.opt()])
            else:
                nc.sync.dma_start(a2a_out[:], a2a_in[:])
            if debug:
                nc.sync.dma_start(dbg_n1, n1_full[:])
                nc.sync.dma_start(dbg_vals, a2a_in[:])
                nc.sync.dma_start(dbg_pv, a2a_out[:])

            # ============ Phase C: proj + residual + LN2 (own chunk) ============
            n2pool = tc.alloc_tile_pool(name="n2pool", bufs=1)
            n2_sb = n2pool.tile([128, NCT, CH], F32R, tag="n2_sb")
            with tc.tile_pool(name="projw", bufs=3) as projw, \
                 tc.tile_pool(name="vf", bufs=4) as vfp, \
                 tc.tile_pool(name="xtr", bufs=2) as xtrp, \
                 tc.tile_pool(name="r1t", bufs=3) as r1tp, \
                 tc.tile_pool(name="lnC_small", bufs=1) as lnCs, \
                 tc.tile_pool(name="psP", bufs=1, space="PSUM") as psP:
                ps_sum2 = psP.tile([128, CH], F32, tag="sum2")
                ps_ssq2 = psP.tile([128, CH], F32, tag="ssq2")
                for oh in range(4):
                    ps_p = [psP.tile([128, CH], F32, tag=f"p{o}", name=f"ps_p{o}") for o in range(4)]
                    for k in range(NCT):
                        vf = vfp.tile([128, CH], F32R, tag="vf")
                        nc.sync.dma_start(
                            vf[:], a2a_out[128 * k:128 * (k + 1), :])
                        wp = projw.tile([128, 512], F32R, tag="wp")
                        nc.sync.dma_start(
                            wp[:], wproj[128 * k:128 * (k + 1),
                                         512 * oh:512 * (oh + 1)])
                        for o in range(4):
                            nc.tensor.matmul(
                                ps_p[o][:], wp[:, 128 * o:128 * (o + 1)],
                                vf[:], start=(k == 0), stop=(k == NCT - 1))
                    for o in range(4):
                        og = 4 * oh + o
                        xtr = xtrp.tile([128, CH], F32, tag="xtr")
                        nc.sync.dma_start(xtr[:], xt[128 * og:128 * (og + 1), :])
                        r1t = r1tp.tile([128, CH], F32, tag="r1t")
                        # (psum + bproj) + x
                        nc.vector.scalar_tensor_tensor(
                            r1t[:], ps_p[o][:], bproj_sb[:, og, :], xtr[:],
                            op0=OP.add, op1=OP.add)
                        sq = r1tp.tile([128, CH], F32, tag="r1sq")
                        _ln_stats_mm(nc, ps_sum2, ps_ssq2, ones_sq, r1t[:],
                                     sq, og, NCT)
                        nc.sync.dma_start(
                            r1_dram[128 * og:128 * (og + 1), :], r1t[:])
                rstd2_bc, shift2_bc = _ln_finish(nc, lnCs, ps_sum2, ps_ssq2, C, CH)
                for k in range(NCT):
                    r1b = r1tp.tile([128, CH], F32, tag="r1b")
                    nc.sync.dma_start(r1b[:],
                                      r1_dram[128 * k:128 * (k + 1), :])
                    nc.vector.tensor_mul(n2_sb[:, k, :], r1b[:], rstd2_bc[:])
                    nc.vector.tensor_add(n2_sb[:, k, :], n2_sb[:, k, :],
                                         shift2_bc[:])

            if debug:
                nc.sync.dma_start(dbg_r1, r1_dram[:])
            # ============ Phase D: FFN (own chunk) ============
            FBLK = 4                     # f-tiles per block
            NFB = NFT // FBLK
            with tc.tile_pool(name="acc2", bufs=1) as acc2p, \
                 tc.tile_pool(name="hblk", bufs=2) as hp, \
                 tc.tile_pool(name="w1", bufs=3) as w1p, \
                 tc.tile_pool(name="w2", bufs=2) as w2p, \
                 tc.tile_pool(name="outp", bufs=3) as outp, \
                 tc.tile_pool(name="psH", bufs=3, space="PSUM") as psH, \
                 tc.tile_pool(name="psF", bufs=3, space="PSUM") as psF:
                acc2 = acc2p.tile([128, NCT, CH], F32, tag="acc2")
                for fb in range(NFB):
                    hblk = hp.tile([128, FBLK, CH], F32R, tag="h")
                    w2rows = []
                    for f4 in range(FBLK):
                        ft = FBLK * fb + f4
                        w1 = w1p.tile([128, NCT, 128], F32R, tag="w1")
                        nc.sync.dma_start(
                            w1[:], wf1t[ft].rearrange("(k p) m -> p k m", p=128))
                        ps_h = psH.tile([128, CH], F32, tag="h")
                        for k in range(NCT):
                            nc.tensor.matmul(
                                ps_h[:], w1[:, k, :], n2_sb[:, k, :],
                                start=(k == 0), stop=(k == NCT - 1))
                        nc.scalar.activation(hblk[:, f4, :], ps_h[:],
                                             GELU,
                                             bias=bf1_sb[:, ft, :], scale=1.0)
                        w2r = w2p.tile([128, C], F32R, tag=f"w2_{f4}")
                        nc.sync.dma_start(
                            w2r[:], wf2[128 * ft:128 * (ft + 1), :])
                        w2rows.append(w2r)
                    for ot in range(NCT):
                        ps_f = psF.tile([128, CH], F32, tag="f")
                        for f4 in range(FBLK):
                            nc.tensor.matmul(
                                ps_f[:], w2rows[f4][:, 128 * ot:128 * (ot + 1)],
                                hblk[:, f4, :],
                                start=(f4 == 0), stop=(f4 == FBLK - 1))
                        if fb == 0:
                            nc.vector.tensor_copy(acc2[:, ot, :], ps_f[:])
                        else:
                            nc.vector.tensor_add(acc2[:, ot, :], acc2[:, ot, :],
                                                 ps_f[:])
                # final: out = (acc2 + bf2) + r1
                for ot in range(NCT):
                    r1b = outp.tile([128, CH], F32, tag="r1_final")
                    nc.sync.dma_start(r1b[:], r1_dram[128 * ot:128 * (ot + 1), :])
                    o_t = outp.tile([128, CH], F32, tag="o_t")
                    nc.vector.scalar_tensor_tensor(
                        o_t[:], acc2[:, ot, :], bf2_sb[:, ot, :], r1b[:],
                        op0=OP.add, op1=OP.add)
                    nc.sync.dma_start(out[128 * ot:128 * (ot + 1), :], o_t[:])
            n2pool.release()

    nc.compile()
    return nc


def _qk_col(o, r, T, CH):
    """Column offset in q_sb/k_sb for head-slot o, token chunk r."""
    return _qk_col_tok(o, r * CH, T, CH)


def _qk_col_tok(h, tok, T, CH):
    """q_sb is [128, H_PER_CORE*B*T] with layout col = h*(B*T) + global_token."""
    return h * (B * T) + tok


# ----------------------------------------------------------------------------
# Host side
# ----------------------------------------------------------------------------

_NC_CACHE = {}


def _get_nc(T=2048):
    if T not in _NC_CACHE:
        _NC_CACHE[T] = build_decoder(T)
    return _NC_CACHE[T]


def round_f32r(a):
    """Round-to-nearest fp32 -> fp32r (11 mantissa bits), matching HW."""
    u = np.ascontiguousarray(a, np.float32).view(np.uint32).astype(np.uint64)
    r = ((u + 0x800) & 0xFFFFF000).astype(np.uint32)
    return r.view(np.float32).reshape(np.asarray(a).shape)


def _prep_inputs(x, Wqkv, bqkv, Wproj, bproj, Wf1, bf1, Wf2, bf2,
                 g1, b1, g2, b2):
    """Fold LN affines, slice heads per core, build per-core in_maps."""
    f32 = np.float32
    x = np.asarray(x, f32)
    Bx, T, Cx = x.shape
    NT = Bx * T
    CH = NT // N_CORES
    Wqkv = np.asarray(Wqkv, f32)
    bqkv = np.asarray(bqkv, f32)
    g1 = np.asarray(g1, f32); b1 = np.asarray(b1, f32)
    g2 = np.asarray(g2, f32); b2 = np.asarray(b2, f32)
    Wqkv_eff = g1[:, None] * Wqkv
    bqkv_eff = b1 @ Wqkv + bqkv
    Wf1 = np.asarray(Wf1, f32)
    bf1v = np.asarray(bf1, f32)
    Wf1_eff = g2[:, None] * Wf1
    bf1_eff = b2 @ Wf1 + bf1v
    Wproj = np.asarray(Wproj, f32)
    bprojv = np.asarray(bproj, f32)
    Wf2 = np.asarray(Wf2, f32)
    bf2v = np.asarray(bf2, f32)

    xt = np.ascontiguousarray(x.reshape(NT, Cx).T)          # [C, NT]
    wf1t = np.ascontiguousarray(
        Wf1_eff.reshape(Cx, NFT, 128).transpose(1, 0, 2))   # [64, C, 128]

    QS = min(512, T)
    masks = np.zeros((128, 4, QS), f32)
    p = np.arange(128)[:, None]
    fcol = np.arange(QS)[None, :]
    for m in range(4):
        masks[:, m, :] = (p <= fcol - 128 * m).astype(f32)

    shared = {
        "wproj": round_f32r(Wproj),
        "bproj": bprojv.reshape(Cx, 1),
        "wf1t": round_f32r(wf1t),
        "bf1": bf1_eff.reshape(F, 1),
        "wf2": round_f32r(Wf2),
        "bf2": bf2v.reshape(Cx, 1),
        "masks": masks,
    }
    in_maps = []
    for c in range(N_CORES):
        h0, h1 = 2 * c, 2 * c + 1
        qcols = np.concatenate([h0 * 384 + np.arange(128),
                                h1 * 384 + np.arange(128)])
        kcols = qcols + 128
        vcols = qcols + 256
        m = dict(shared)
        m["xt"] = np.ascontiguousarray(xt[:, c * CH:(c + 1) * CH])
        m["wq"] = round_f32r(Wqkv_eff[:, qcols])
        m["wk"] = round_f32r(Wqkv_eff[:, kcols])
        m["wv"] = round_f32r(Wqkv_eff[:, vcols])
        m["bq"] = np.ascontiguousarray(bqkv_eff[qcols].reshape(256, 1))
        m["bk"] = np.ascontiguousarray(bqkv_eff[kcols].reshape(256, 1))
        m["bv_bc"] = np.ascontiguousarray(
            np.broadcast_to(bqkv_eff[vcols][None, :], (128, 256)))
        in_maps.append(m)
    return in_maps, (Bx, T, Cx, CH)


def kernel(x, Wqkv, bqkv, Wproj, bproj, Wf1, bf1, Wf2, bf2,
           g1, b1, g2, b2, _trace=False):
    in_maps, (Bx, T, Cx, CH) = _prep_inputs(
        x, Wqkv, bqkv, Wproj, bproj, Wf1, bf1, Wf2, bf2, g1, b1, g2, b2)
    nc = _get_nc(T)
    res = bass_utils.run_bass_kernel_spmd(
        nc, in_maps, core_ids=list(range(N_CORES)), trace=_trace)
    kernel.last_results = res
    NT = Bx * T
    out_t = np.empty((NT, Cx), np.float32)
    for c in range(N_CORES):
        out_t[c * CH:(c + 1) * CH, :] = res.results[c]["out"].T
    return out_t.reshape(Bx, T, Cx)



# revision 30
# speedup vs baseline: 2.6599x; 2.6599x over previous
"""Trainium2 Bass kernel for a dense decoder block (LN->MHA->res, LN->FFN->res).

Sharding (8 cores, one NEFF, SPMD-uniform addressing):
  - LN1 token-parallel (512-token chunk/core) -> AllGather of normalized acts.
  - QKV + attention head-parallel (2 heads/core, causal, unstable softmax --
    exact because masked logits multiply to 0 post-exp).
  - AllToAll redistributes attention values: head-shards -> token-shards.
  - proj + residual + LN2 + FFN token-parallel with full weights streamed.
  - LN affine params are folded into the following matmul weights on host.

Speed strategy: weights and most activations are quantized to fp8-e4m3 and
all big matmuls run in DoubleRow perf mode (contraction pairs of 128-channel
planes, 0.5 PE cycles per output row).  Attention scores stay f32r for
accuracy.  Elementwise work is spread over DVE / Pool / Act engines.
"""

import math

import numpy as np
import ml_dtypes

import concourse.bass as bass
import concourse.mybir as mybir
import concourse.tile as tile
from concourse import bacc
from concourse import bass_utils

F32 = mybir.dt.float32
F32R = mybir.dt.float32r
F8 = mybir.dt.float8e4
F8NP = ml_dtypes.float8_e4m3
AF = mybir.ActivationFunctionType
OP = mybir.AluOpType
DR = mybir.MatmulPerfMode.DoubleRow
BF16 = mybir.dt.bfloat16

N_CORES = 8
B = 2
C = 2048
H = 16
HD = 128
F = 8192
H_PER_CORE = H // N_CORES          # 2
NCT = C // 128                     # 16 channel tiles
NDK = C // 256                     # 8 channel pair-planes
NF2 = F // 256                     # 32 ffn pair-planes
EPS = 1e-5
SCALE = 1.0 / math.sqrt(HD)
GELU = AF.Gelu_apprx_tanh
WS1 = 32.0     # fp8 scale for fan-in-C weights (moves into e4m3 normal range)
WS2 = 64.0     # fp8 scale for Wf2 (fan-in F)
EXPB = 2.0     # exp bias: e' = e^(s*SCALE+2), cancels in av/den ratio

# which weights ship as hi+lo fp8 pairs (error compensation)
COMP_W = {"wqkv": False, "wproj": True, "wf1": False, "wf2": False}
# compensated activations
COMP_N2 = False
COMP_H = False


def r32(ap):
    return ap.bitcast(F32R)


def _ln_finish(nc, pool_small, ps_sum, ps_ssq, n_tok, ncols, tag):
    """From broadcast sum/sumsq psums produce SBUF rstd/shift [128, ncols]."""
    mean = pool_small.tile([128, ncols], F32, tag=f"{tag}_mean")
    ex2 = pool_small.tile([128, ncols], F32, tag=f"{tag}_ex2")
    nc.vector.tensor_scalar_mul(mean[:], ps_sum[:], 1.0 / n_tok)
    nc.vector.tensor_scalar_mul(ex2[:], ps_ssq[:], 1.0 / n_tok)
    msq = pool_small.tile([128, ncols], F32, tag=f"{tag}_msq")
    nc.vector.tensor_mul(msq[:], mean[:], mean[:])
    varp = pool_small.tile([128, ncols], F32, tag=f"{tag}_varp")
    nc.vector.scalar_tensor_tensor(varp[:], ex2[:], EPS, msq[:],
                                   op0=OP.add, op1=OP.subtract)
    std = pool_small.tile([128, ncols], F32, tag=f"{tag}_std")
    nc.scalar.sqrt(std[:], varp[:])
    rstd_bc = pool_small.tile([128, ncols], F32, tag=f"{tag}_rstd")
    nc.vector.reciprocal(rstd_bc[:], std[:])
    shift_bc = pool_small.tile([128, ncols], F32, tag=f"{tag}_shift")
    nc.vector.scalar_tensor_tensor(shift_bc[:], mean[:], -1.0, rstd_bc[:],
                                   op0=OP.mult, op1=OP.mult)
    return rstd_bc, shift_bc


def build_decoder(T=2048, collectives=True, debug=False):
    """Build the SPMD decoder-block program for seq length T (2048 = real)."""
    NT = B * T                      # total tokens
    CH = NT // N_CORES              # tokens per core chunk (512)
    NTT = NT // 128                 # global token tiles (32)
    QS = 512                        # query slice width
    NQS = T // QS                   # query slices per batch elem (4)

    nc = bacc.Bacc("TRN2", target_bir_lowering=False, debug=False,
                   num_devices=N_CORES)

    # ---- I/O ----
    xt = nc.dram_tensor("xt", [C, CH], F32R, kind="ExternalInput").ap()
    wq = nc.dram_tensor("wq", [128, NDK * 2 * 256], F8, kind="ExternalInput").ap()
    wk = nc.dram_tensor("wk", [128, NDK * 2 * 256], F8, kind="ExternalInput").ap()
    wv = nc.dram_tensor("wv", [128, NDK * 2 * 256], F8, kind="ExternalInput").ap()
    bq = nc.dram_tensor("bq", [256, 1], F32, kind="ExternalInput").ap()
    bk = nc.dram_tensor("bk", [256, 1], F32, kind="ExternalInput").ap()
    bv_bc = nc.dram_tensor("bv_bc", [128, 256], F32, kind="ExternalInput").ap()
    n_wp = 2 if COMP_W["wproj"] else 1
    wproj = nc.dram_tensor("wproj", [n_wp, NDK, 128, 2, C], F8,
                           kind="ExternalInput").ap()
    bproj = nc.dram_tensor("bproj", [C, 1], F32, kind="ExternalInput").ap()
    n_w1 = 2 if COMP_W["wf1"] else 1
    wf1 = nc.dram_tensor("wf1", [n_w1, F // 128, 128, NDK * 2 * 128], F8,
                         kind="ExternalInput").ap()
    bf1 = nc.dram_tensor("bf1", [F, 1], F32, kind="ExternalInput").ap()
    n_w2 = 2 if COMP_W["wf2"] else 1
    wf2 = nc.dram_tensor("wf2", [n_w2, NF2, 128, 2, C], F8,
                         kind="ExternalInput").ap()
    bf2 = nc.dram_tensor("bf2", [C, 1], F32, kind="ExternalInput").ap()
    masks = nc.dram_tensor("masks", [128, 2, 2, QS], F8, kind="ExternalInput").ap()
    out = nc.dram_tensor("out", [C, CH], F32, kind="ExternalOutput").ap()

    RG = [list(range(N_CORES))]

    def qk_col(h, tok):
        """q_sb/k_sb layout: col = h*(B*T) + global_token."""
        return h * NT + tok

    with tile.TileContext(nc) as tc:
        with tc.tile_pool(name="dram", bufs=1, space="DRAM") as dram, \
             tc.tile_pool(name="persist", bufs=1) as persist:
            n1_bounce = dram.tile([C, CH], F8, tag="n1_bounce")
            n1_full = dram.tile([N_CORES * C, CH], F8, tag="n1_full",
                                addr_space="Shared")
            a2a_in = dram.tile([C, CH], F8, tag="a2a_in")
            a2a_out = dram.tile([C, CH], F8, tag="a2a_out")

            ones_r = persist.tile([128, 128], F32R, tag="ones_r")
            ones_f = persist.tile([128, 128], F32, tag="ones_f")
            nc.vector.memset(ones_f[:], 1.0)
            nc.vector.tensor_copy(ones_r[:], ones_f[:])
            ones8 = persist.tile([128, 2, 128], F8, tag="ones8")
            ones8f = persist.tile([128, 2, 128], F32, tag="ones8f")
            nc.vector.memset(ones8f[:], 1.0)
            nc.vector.tensor_copy(ones8[:], ones8f[:])
            expb_sb = persist.tile([128, 1], F32, tag="expb")
            nc.vector.memset(expb_sb[:], EXPB)
            masks_sb = persist.tile([128, 4, QS], F8, tag="masks")
            nc.sync.dma_start(masks_sb[:], masks)
            bq_sb = persist.tile([128, 2, 1], F32, tag="bq")
            bk_sb = persist.tile([128, 2, 1], F32, tag="bk")
            nc.sync.dma_start(bq_sb[:], bq.rearrange("(o p) u -> p o u", p=128))
            nc.sync.dma_start(bk_sb[:], bk.rearrange("(o p) u -> p o u", p=128))
            bv_sb = persist.tile([128, 256], F32, tag="bv")
            nc.sync.dma_start(bv_sb[:], bv_bc)
            bproj_sb = persist.tile([128, NCT, 1], F32, tag="bproj")
            nc.sync.dma_start(bproj_sb[:], bproj.rearrange("(o p) u -> p o u", p=128))
            bf1_sb = persist.tile([128, F // 128, 1], F32, tag="bf1")
            nc.sync.dma_start(bf1_sb[:], bf1.rearrange("(o p) u -> p o u", p=128))
            bf2_sb = persist.tile([128, NCT, 1], F32, tag="bf2")
            nc.sync.dma_start(bf2_sb[:], bf2.rearrange("(o p) u -> p o u", p=128))

            xpool = tc.alloc_tile_pool(name="xpool", bufs=1)
            x_sb = xpool.tile([128, NCT, CH], F32R, tag="x_sb")
            for k in range(NCT):
                nc.sync.dma_start(x_sb[:, k, :], xt[128 * k:128 * (k + 1), :])

            # ================= Phase A: LN1 on own chunk =================
            with tc.tile_pool(name="lnA", bufs=3) as lnA, \
                 tc.tile_pool(name="lnA_small", bufs=1) as lnAs, \
                 tc.tile_pool(name="n1pool", bufs=1) as n1pool, \
                 tc.tile_pool(name="psA", bufs=1, space="PSUM") as psA:
                ps_sum = psA.tile([128, CH], F32, tag="sum")
                ps_ssq = psA.tile([128, CH], F32, tag="ssq")
                for k in range(NCT):
                    sq = lnA.tile([128, CH], F32R, tag="sq")
                    nc.vector.tensor_mul(sq[:], x_sb[:, k, :], x_sb[:, k, :])
                    nc.tensor.matmul(ps_sum[:], ones_r[:], x_sb[:, k, :],
                                     start=(k == 0), stop=(k == NCT - 1))
                    nc.tensor.matmul(ps_ssq[:], ones_r[:], sq[:],
                                     start=(k == 0), stop=(k == NCT - 1))
                rstd_bc, shift_bc = _ln_finish(nc, lnAs, ps_sum, ps_ssq, C, CH, "ln1")
                n1_sb = n1pool.tile([128, NCT, CH], F8, tag="n1_sb")
                for k in range(NCT):
                    t1 = lnA.tile([128, CH], F32, tag="t1")
                    nc.vector.tensor_mul(t1[:], x_sb[:, k, :], rstd_bc[:])
                    nc.vector.scalar_tensor_tensor(n1_sb[:, k, :], t1[:], 1.0,
                                                   shift_bc[:], op0=OP.mult,
                                                   op1=OP.add)
                for k2 in range(NCT // 2):
                    nc.sync.dma_start(
                        n1_bounce[256 * k2:256 * (k2 + 1), :]
                        .rearrange("(k p) t -> p k t", p=128),
                        n1_sb[:, 2 * k2:2 * k2 + 2, :])

            if collectives:
                nc.gpsimd.collective_compute(
                    "AllGather", OP.bypass, replica_groups=RG,
                    ins=[n1_bounce.opt()], outs=[n1_full.opt()])
            else:  # timing variant: plain copy keeps the dependency edge
                nc.sync.dma_start(n1_full[0:C, :], n1_bounce[:])

            # ============ Phase B: QKV (all tokens, own 2 heads) ============
            with tc.tile_pool(name="wqkv", bufs=1) as wqkvp, \
                 tc.tile_pool(name="qkv_sb", bufs=1) as qkvp:
                wq_sb = wqkvp.tile([128, NDK, 2, 256], F8, tag="wq")
                wk_sb = wqkvp.tile([128, NDK, 2, 256], F8, tag="wk")
                wv_sb = wqkvp.tile([128, NDK, 2, 256], F8, tag="wv")
                nc.sync.dma_start(wq_sb[:], wq.rearrange(
                    "p (dk pl m) -> p dk pl m", dk=NDK, pl=2))
                nc.sync.dma_start(wk_sb[:], wk.rearrange(
                    "p (dk pl m) -> p dk pl m", dk=NDK, pl=2))
                nc.sync.dma_start(wv_sb[:], wv.rearrange(
                    "p (dk pl m) -> p dk pl m", dk=NDK, pl=2))
                q_sb = qkvp.tile([128, H_PER_CORE * NT], BF16, tag="q_sb")
                k_sb = qkvp.tile([128, H_PER_CORE * NT], BF16, tag="k_sb")
                v_sb = qkvp.tile([128, NTT, 256], F8, tag="v_sb")

                with tc.tile_pool(name="n1t", bufs=18) as n1tp, \
                     tc.tile_pool(name="attn_e", bufs=4) as ep, \
                     tc.tile_pool(name="attn_small", bufs=3) as asml, \
                     tc.tile_pool(name="vals", bufs=2) as valsp, \
                     tc.tile_pool(name="psQK", bufs=1, space="PSUM") as psQK, \
                     tc.tile_pool(name="psV", bufs=2, space="PSUM") as psV, \
                     tc.tile_pool(name="psS", bufs=2, space="PSUM") as psS, \
                     tc.tile_pool(name="psAD", bufs=1, space="PSUM") as psAD:

                    def emit_qkv_chunk(r):
                        n1ts = []
                        for dk2 in range(NDK // 2):
                            n1t4 = n1tp.tile([128, 4, CH], F8, tag="n1t",
                                             name=f"n1t_{r}_{dk2}")
                            nc.sync.dma_start(
                                n1t4[:],
                                n1_full[C * r + 512 * dk2:
                                        C * r + 512 * (dk2 + 1), :]
                                .rearrange("(pl p) t -> p pl t", pl=4))
                            n1ts.append(n1t4[:, 0:2, :])
                            n1ts.append(n1t4[:, 2:4, :])
                        for qk in range(2):
                            ps_qk = psQK.tile([128, 2, CH], F32, tag="qk",
                                              name=f"ps_qk_{r}_{qk}")
                            w_sb = wq_sb if qk == 0 else wk_sb
                            for o in range(2):
                                for dk in range(NDK):
                                    nc.tensor.matmul(
                                        ps_qk[:, o, :],
                                        w_sb[:, dk, :, 128 * o:128 * (o + 1)],
                                        n1ts[dk][:], start=(dk == 0),
                                        stop=(dk == NDK - 1), perf_mode=DR)
                                col = qk_col(o, r * CH)
                                if qk == 0:
                                    nc.scalar.activation(
                                        q_sb[:, col:col + CH], ps_qk[:, o, :],
                                        AF.Copy, bias=bq_sb[:, o, :],
                                        scale=1.0 / WS1)
                                else:
                                    nc.vector.tensor_scalar(
                                        k_sb[:, col:col + CH], ps_qk[:, o, :],
                                        1.0 / WS1, bk_sb[:, o, :],
                                        op0=OP.mult, op1=OP.add)
                        for sv2 in range(2):
                            ps_v = psV.tile([128, 2, 256], F32, tag="v",
                                            name=f"ps_v_{r}_{sv2}")
                            for s2 in range(2):
                                sv = 2 * sv2 + s2
                                for dk in range(NDK):
                                    nc.tensor.matmul(
                                        ps_v[:, s2, :],
                                        n1ts[dk][:, :, 128 * sv:128 * (sv + 1)]
                                        if True else None,
                                        wv_sb[:, dk, :, :],
                                        start=(dk == 0), stop=(dk == NDK - 1),
                                        perf_mode=DR)
                                nc.vector.scalar_tensor_tensor(
                                    v_sb[:, 4 * r + sv, :], ps_v[:, s2, :],
                                    1.0 / WS1, bv_sb[:], op0=OP.mult,
                                    op1=OP.add)

                    def emit_attn(h, bb, j):
                        npair = 2 * (j + 1)
                        ps_ad = psAD.tile([128, 2, QS], F32, tag="ad",
                                          name=f"ps_ad_{h}_{bb}_{j}")
                        qcol = qk_col(h, bb * T + j * QS)
                        for pi in range(npair):
                            ps_s = psS.tile([128, 2, QS], F32, tag="s",
                                            name=f"ps_s_{h}_{bb}_{j}_{pi}")
                            for l in range(2):
                                kcol = qk_col(h, bb * T + (2 * pi + l) * 128)
                                nc.tensor.matmul(
                                    ps_s[:, l, :], k_sb[:, kcol:kcol + 128],
                                    q_sb[:, qcol:qcol + QS],
                                    start=True, stop=True)
                            e = ep.tile([128, 2, QS], F8, tag="e",
                                        name=f"e_{h}_{bb}_{j}_{pi}")
                            nc.scalar.activation(e[:], ps_s[:], AF.Exp,
                                                 bias=expb_sb[:], scale=SCALE)
                            g = pi - (npair - 2)
                            if g >= 0:
                                nc.vector.tensor_mul(
                                    e[:], e[:], masks_sb[:, 2 * g:2 * g + 2, :])
                            nc.tensor.matmul(
                                ps_ad[:, 1, :], ones8[:], e[:],
                                start=(pi == 0), stop=(pi == npair - 1),
                                perf_mode=DR)
                            tt0 = bb * (T // 128) + 2 * pi
                            nc.tensor.matmul(
                                ps_ad[:, 0, :],
                                v_sb[:, tt0:tt0 + 2, 128 * h:128 * (h + 1)],
                                e[:],
                                start=(pi == 0), stop=(pi == npair - 1),
                                perf_mode=DR)
                        rec_bc = asml.tile([128, QS], F32, tag="rec_bc",
                                           name=f"rec_{h}_{bb}_{j}")
                        nc.vector.reciprocal(rec_bc[:], ps_ad[:, 1, :])
                        vtile = valsp.tile([128, QS], F8, tag="vt",
                                           name=f"vt_{h}_{bb}_{j}")
                        nc.vector.tensor_mul(vtile[:], ps_ad[:, 0, :], rec_bc[:])
                        jg = bb * NQS + j
                        nc.sync.dma_start(
                            a2a_in[256 * jg + 128 * h:
                                   256 * jg + 128 * (h + 1), :],
                            vtile[:])

                    for r in range(4):
                        emit_qkv_chunk(r)
                    for j in range(NQS):
                        for h in range(H_PER_CORE):
                            emit_attn(h, 0, j)
                        emit_qkv_chunk(4 + j)
                    for j in range(NQS):
                        for h in range(H_PER_CORE):
                            emit_attn(h, 1, j)

            if collectives:
                nc.gpsimd.collective_compute(
                    "AllToAll", OP.bypass, replica_groups=RG,
                    ins=[a2a_in.opt()], outs=[a2a_out.opt()])
            else:
                nc.sync.dma_start(a2a_out[:], a2a_in[:])

            # ============ Phase C: proj + residual + LN2 (own chunk) ============
            n2pool = tc.alloc_tile_pool(name="n2pool", bufs=1)
            n2_sb = n2pool.tile([128, NCT, CH], F8, tag="n2_sb")
            if COMP_N2:
                n2lo_sb = n2pool.tile([128, NCT, CH], F8, tag="n2lo_sb")
            r1pool = tc.alloc_tile_pool(name="r1pool", bufs=1)
            r1_sb = r1pool.tile([128, NCT, CH], F32R, tag="r1_sb")
            with tc.tile_pool(name="vfp", bufs=1) as vfp, \
                 tc.tile_pool(name="projw", bufs=2) as projw, \
                 tc.tile_pool(name="lnC", bufs=3) as lnC, \
                 tc.tile_pool(name="lnC_small", bufs=1) as lnCs, \
                 tc.tile_pool(name="psP", bufs=2, space="PSUM") as psP:
                # wp tiles cover an oh-pair of columns; first half emitted
                # before the a2a wait so its DMA overlaps attention
                def load_wp(ohp):
                    wps = []
                    for w_i in range(n_wp):
                        for dk in range(NDK):
                            wp = projw.tile([128, 2, 1024], F8,
                                            tag=f"wp{w_i}_{dk}",
                                            name=f"wp{ohp}_{w_i}_{dk}")
                            nc.sync.dma_start(
                                wp[:], wproj[w_i, dk][:, :,
                                             1024 * ohp:1024 * (ohp + 1)])
                            wps.append((w_i, dk, wp))
                    return wps
                wps_half = load_wp(0)
                vf_sb = vfp.tile([128, NDK, 2, CH], F8, tag="vf_sb")
                for dk in range(NDK):
                    nc.sync.dma_start(
                        vf_sb[:, dk],
                        a2a_out[256 * dk:256 * (dk + 1), :]
                        .rearrange("(pl p) t -> p pl t", pl=2))
                ps_sum2 = psP.tile([128, CH], F32, tag="sum2")
                ps_ssq2 = psP.tile([128, CH], F32, tag="ssq2")
                for oh in range(4):
                    if oh == 2:
                        wps_half = load_wp(1)
                    coff = 512 * (oh % 2)
                    for op2 in range(2):
                        ps_p = psP.tile([128, 2, CH], F32, tag="pp", name="ps_p")
                        for w_i, dk, wp in wps_half:
                            for o2 in range(2):
                                o = 2 * op2 + o2
                                nc.tensor.matmul(
                                    ps_p[:, o2, :],
                                    wp[:, :, coff + 128 * o:coff + 128 * (o + 1)],
                                    vf_sb[:, dk],
                                    start=(w_i == 0 and dk == 0),
                                    stop=(dk == NDK - 1 and w_i == n_wp - 1),
                                    perf_mode=DR)
                        for o2 in range(2):
                            og = 4 * oh + 2 * op2 + o2
                            tp = lnC.tile([128, CH], F32, tag="tp")
                            nc.scalar.activation(
                                tp[:], ps_p[:, o2, :], AF.Identity,
                                bias=bproj_sb[:, og, :], scale=1.0 / WS1)
                            nc.vector.tensor_add(r1_sb[:, og, :], tp[:],
                                                 x_sb[:, og, :])
                            sq = lnC.tile([128, CH], F32R, tag="r1sq")
                            nc.scalar.square(sq[:], r1_sb[:, og, :])
                            nc.tensor.matmul(ps_sum2[:], ones_r[:],
                                             r1_sb[:, og, :],
                                             start=(og == 0), stop=(og == NCT - 1))
                            nc.tensor.matmul(ps_ssq2[:], ones_r[:], sq[:],
                                             start=(og == 0), stop=(og == NCT - 1))
                rstd2, shift2 = _ln_finish(nc, lnCs, ps_sum2, ps_ssq2, C, CH, "ln2")
                for k in range(NCT):
                    t2 = lnC.tile([128, CH], F32, tag="t2")
                    nc.vector.tensor_mul(t2[:], r1_sb[:, k, :], rstd2[:])
                    nc.vector.scalar_tensor_tensor(
                        n2_sb[:, k, :], t2[:], 1.0, shift2[:],
                        op0=OP.mult, op1=OP.add)
                    if COMP_N2:
                        tsub = lnC.tile([128, CH], F32, tag="tsub")
                        nc.vector.tensor_sub(tsub[:], t2[:], n2_sb[:, k, :])
                        nc.vector.scalar_tensor_tensor(
                            n2lo_sb[:, k, :], tsub[:], 1.0, shift2[:],
                            op0=OP.mult, op1=OP.add)

            # ============ Phase D: FFN (own chunk) ============
            # pass list: (weight_index, activation_index); skip lo*lo cross
            n2_list = [n2_sb] + ([n2lo_sb] if COMP_N2 else [])
            ffn1_passes = [(w_i, a_i) for w_i in range(n_w1)
                           for a_i in range(len(n2_list))
                           if not (w_i == 1 and a_i == 1)]
            hpool = tc.alloc_tile_pool(name="hpool", bufs=1)
            h_sb = hpool.tile([128, NF2, 2, CH], F8, tag="h_sb")
            if COMP_H:
                hlo_sb = hpool.tile([128, NF2, 2, CH], F8, tag="hlo_sb")
            with tc.tile_pool(name="w1", bufs=3) as w1p, \
                 tc.tile_pool(name="gtmp", bufs=3) as gtmp, \
                 tc.tile_pool(name="psH", bufs=2, space="PSUM") as psH:
                for fp in range(NF2):
                    ps_h = psH.tile([128, 2, CH], F32, tag="h")
                    for l in range(2):
                        ft = 2 * fp + l
                        w1ts = []
                        for w_i in range(n_w1):
                            w1t = w1p.tile([128, NDK, 2, 128], F8, tag=f"w1_{w_i}",
                                           name=f"w1t{w_i}")
                            q1 = (nc.sync, nc.scalar, nc.gpsimd)[ft % 3]
                            q1.dma_start(
                                w1t[:], wf1[w_i, ft].rearrange(
                                    "p (dk pl m) -> p dk pl m", dk=NDK, pl=2))
                            w1ts.append(w1t)
                        for dk in range(NDK):
                            for p_i, (w_i, a_i) in enumerate(ffn1_passes):
                                nc.tensor.matmul(
                                    ps_h[:, l, :], w1ts[w_i][:, dk],
                                    n2_list[a_i][:, 2 * dk:2 * dk + 2, :],
                                    start=(dk == 0 and p_i == 0),
                                    stop=(dk == NDK - 1
                                          and p_i == len(ffn1_passes) - 1),
                                    perf_mode=DR)
                        if COMP_H:
                            g32 = gtmp.tile([128, CH], F32, tag="g32")
                            nc.scalar.activation(g32[:], ps_h[:, l, :], GELU,
                                                 bias=bf1_sb[:, ft, :],
                                                 scale=1.0 / WS1)
                            nc.vector.tensor_copy(h_sb[:, fp, l, :], g32[:])
                            nc.gpsimd.tensor_sub(hlo_sb[:, fp, l, :], g32[:],
                                                 h_sb[:, fp, l, :])
                        else:
                            nc.scalar.activation(h_sb[:, fp, l, :], ps_h[:, l, :],
                                                 GELU, bias=bf1_sb[:, ft, :],
                                                 scale=1.0 / WS1)

            h_list = [h_sb] + ([hlo_sb] if COMP_H else [])
            ffn2_passes = [(w_i, a_i) for w_i in range(n_w2)
                           for a_i in range(len(h_list))
                           if not (w_i == 1 and a_i == 1)]
            with tc.tile_pool(name="w2", bufs=4) as w2p, \
                 tc.tile_pool(name="outp", bufs=3) as outp, \
                 tc.tile_pool(name="psF", bufs=2, space="PSUM") as psF:
                for qtr in range(4):
                    ps_f = [psF.tile([128, CH], F32, tag=f"f{oo}",
                                     name=f"ps_f{qtr}_{oo}")
                            for oo in range(4)]
                    for fp in range(NF2):
                        w2ts = []
                        for w_i in range(n_w2):
                            w2t = w2p.tile([128, 2, 512], F8, tag=f"w2_{w_i}",
                                           name=f"w2t{qtr}_{fp}_{w_i}")
                            q2 = (nc.sync, nc.scalar, nc.gpsimd)[fp % 3]
                            q2.dma_start(
                                w2t[:], wf2[w_i, fp][:, :,
                                             512 * qtr:512 * (qtr + 1)])
                            w2ts.append(w2t)
                        for oo in range(4):
                            for p_i, (w_i, a_i) in enumerate(ffn2_passes):
                                nc.tensor.matmul(
                                    ps_f[oo][:],
                                    w2ts[w_i][:, :, 128 * oo:128 * (oo + 1)],
                                    h_list[a_i][:, fp],
                                    start=(fp == 0 and p_i == 0),
                                    stop=(fp == NF2 - 1
                                          and p_i == len(ffn2_passes) - 1),
                                    perf_mode=DR)
                    for oo in range(4):
                        og = 4 * qtr + oo
                        t2o = outp.tile([128, CH], F32, tag="t2o")
                        nc.scalar.activation(
                            t2o[:], ps_f[oo][:], AF.Identity,
                            bias=bf2_sb[:, og, :], scale=1.0 / WS2)
                        o_t = outp.tile([128, CH], F32, tag="o_t")
                        nc.vector.tensor_add(o_t[:], t2o[:],
                                             r1_sb[:, og, :])
                        nc.sync.dma_start(out[128 * og:128 * (og + 1), :],
                                          o_t[:])
            hpool.release()
            r1pool.release()
            n2pool.release()
            xpool.release()

    nc.compile()
    return nc


# ----------------------------------------------------------------------------
# Host side
# ----------------------------------------------------------------------------

_NC_CACHE = {}


def _get_nc(T=2048):
    if T not in _NC_CACHE:
        _NC_CACHE[T] = build_decoder(T)
    return _NC_CACHE[T]


def round_f32r(a):
    """Round-to-nearest fp32 -> fp32r (11 mantissa bits), matching HW."""
    u = np.ascontiguousarray(a, np.float32).view(np.uint32).astype(np.uint64)
    r = ((u + 0x800) & 0xFFFFF000).astype(np.uint32)
    return r.view(np.float32).reshape(np.asarray(a).shape)


def _f8(a, scale):
    return (np.ascontiguousarray(a, np.float32) * scale).astype(F8NP)


def _f8_pair(a, comp, scale):
    """Quantize a*scale to fp8; if comp, return [hi, lo] stack else [hi]."""
    a = np.ascontiguousarray(a, np.float32) * scale
    hi = a.astype(F8NP)
    if not comp:
        return hi[None]
    lo = (a - hi.astype(np.float32)).astype(F8NP)
    return np.stack([hi, lo])


def _pack_ch_pairs(Wst, M):
    """[C, M] -> [128, NDK*2*M] with [p, (dk, pl, m)] = W[256dk+128pl+p, m]."""
    W = Wst.reshape(Wst.shape[0], NDK, 2, 128, M)       # [S?, dk, pl, p, m]
    return W


def kernel(x, Wqkv, bqkv, Wproj, bproj, Wf1, bf1, Wf2, bf2,
           g1, b1, g2, b2, _trace=False):
    f32 = np.float32
    x = np.asarray(x, f32)
    Bx, T, Cx = x.shape
    NT = Bx * T
    CH = NT // N_CORES
    Wqkv = np.asarray(Wqkv, f32)
    bqkv = np.asarray(bqkv, f32)
    g1 = np.asarray(g1, f32); b1 = np.asarray(b1, f32)
    g2 = np.asarray(g2, f32); b2 = np.asarray(b2, f32)
    Wqkv_eff = g1[:, None] * Wqkv
    bqkv_eff = b1 @ Wqkv + bqkv
    Wf1 = np.asarray(Wf1, f32)
    Wf1_eff = g2[:, None] * Wf1
    bf1_eff = b2 @ Wf1 + np.asarray(bf1, f32)
    Wproj = np.asarray(Wproj, f32)
    bprojv = np.asarray(bproj, f32)
    Wf2 = np.asarray(Wf2, f32)
    bf2v = np.asarray(bf2, f32)

    xt = round_f32r(np.ascontiguousarray(x.reshape(NT, Cx).T))  # [C, NT] f32r

    # --- packed fp8 weights ---
    # wproj: [n, NDK, 128, 2, C]: [., dk, p, pl, c] = W[256dk+128pl+p, c]
    wp8 = _f8_pair(Wproj, COMP_W["wproj"], WS1)                  # [n, C, C]
    wp8 = wp8.reshape(-1, NDK, 2, 128, Cx).transpose(0, 1, 3, 2, 4)
    wp8 = np.ascontiguousarray(wp8)
    # wf1: [n, 64, 128, NDK*2*128]: [., ft, p, (dk, pl, m)] =
    #      W[256dk+128pl+p, 128ft+m]
    w18 = _f8_pair(Wf1_eff, COMP_W["wf1"], WS1)                  # [n, C, F]
    w18 = w18.reshape(-1, NDK, 2, 128, F // 128, 128)       # n dk pl p ft m
    w18 = np.ascontiguousarray(w18.transpose(0, 4, 3, 1, 2, 5).reshape(
        -1, F // 128, 128, NDK * 2 * 128))
    # wf2: [n, NF2, 128, 2, C]: [., f2, p, pl, c] = W[256f2+128pl+p, c]
    w28 = _f8_pair(Wf2, COMP_W["wf2"], WS2)                      # [n, F, C]
    w28 = w28.reshape(-1, NF2, 2, 128, Cx).transpose(0, 1, 3, 2, 4)
    w28 = np.ascontiguousarray(w28)

    QS = 512
    masks = np.zeros((128, 2, 2, QS), F8NP)
    p = np.arange(128)[:, None]
    fcol = np.arange(QS)[None, :]
    for g in range(2):
        for l in range(2):
            masks[:, g, l, :] = (p <= fcol - 128 * (2 * g + l)).astype(F8NP)

    shared = {
        "wproj": wp8,
        "bproj": bprojv.reshape(Cx, 1),
        "wf1": w18,
        "bf1": bf1_eff.reshape(F, 1),
        "wf2": w28,
        "bf2": bf2v.reshape(Cx, 1),
        "masks": masks,
    }
    in_maps = []
    for c in range(N_CORES):
        h0, h1 = 2 * c, 2 * c + 1
        qcols = np.concatenate([h0 * 384 + np.arange(128),
                                h1 * 384 + np.arange(128)])
        kcols = qcols + 128
        vcols = qcols + 256
        m = dict(shared)
        m["xt"] = np.ascontiguousarray(xt[:, c * CH:(c + 1) * CH])

        def packqkv(cols):
            W = _f8(Wqkv_eff[:, cols], WS1)                      # [C, 256]
            W = W.reshape(NDK, 2, 128, 256).transpose(2, 0, 1, 3)
            return np.ascontiguousarray(W.reshape(128, NDK * 2 * 256))

        m["wq"] = packqkv(qcols)
        m["wk"] = packqkv(kcols)
        m["wv"] = packqkv(vcols)
        m["bq"] = np.ascontiguousarray(bqkv_eff[qcols].reshape(256, 1))
        m["bk"] = np.ascontiguousarray(bqkv_eff[kcols].reshape(256, 1))
        m["bv_bc"] = np.ascontiguousarray(
            np.broadcast_to(bqkv_eff[vcols][None, :], (128, 256)))
        in_maps.append(m)

    nc = _get_nc(T)
    res = bass_utils.run_bass_kernel_spmd(
        nc, in_maps, core_ids=list(range(N_CORES)), trace=_trace)
    kernel.last_results = res
    out_t = np.empty((NT, Cx), np.float32)
    for c in range(N_CORES):
        out_t[c * CH:(c + 1) * CH, :] = res.results[c]["out"].T
    return out_t.reshape(Bx, T, Cx)
